# revision 11
# baseline (speedup 1.0000x reference)
"""Grouped SwiGLU expert FFN (MoE) on 8 Trainium2 NeuronCores.

Expert parallelism: expert e's weights + its (pre-sorted) token slice go to
core e. Each core runs x@w1, x@w3, silu/mul, h@w2 for its 8192 tokens.

All matmul operands are converted to bf16 on-chip (PSUM accumulation stays
fp32; bf16 runs the PE at the same 1 row/cycle as f32r but without the
~14ns/matmul issue tax fp32r pays). The x transpose is done by the DMA XBAR
(four [128, 1024] bf16 SBUF->SBUF transposes per 512-token block), so the
PE runs matmuls only. Transpose DMAs are dispatched from the scalar (ACT)
HWDGE queue so their input-sem waits never block the sync queue's weight/x
loads; the x pipeline (DMA -> DVE bf16 convert -> XBAR transpose) runs two
blocks ahead.

Math per core (dims: t=tokens, i=dim_in, j=dim_hid, o=dim_in):
  mm1/mm3: psum[j,t] += lhsT=w{1,3}[i_chunk, j_chunk] (stationary, natural
           layout), rhs=xT[i_chunk, t_block] (moving, 512 wide) -> h1T/h3T.
  SwiGLU:  hT_j = silu(h1T) * h3T  (ACT Silu -> s1, DVE mul fuses PSUM evict;
           per-j tiles so mm2's first weight load only waits on mul(j)).
  mm2:     lhsT=hT_j[t_chunk] (stationary), rhs=w2[j_chunk, o_half]
           (moving, 512 wide) -> psum[t,o] natural-layout fp32 output.

A single 8-slot PSUM pool rotates banks through mm1/mm3/mm2 accumulators.
"""

import sys

sys.path.insert(0, "/opt/trn_rl_repo")

import numpy as np

N_CORES = 8
D = 1024  # dim_in
H = 1024  # dim_hid
P = 128
TB = 512  # token block per pipeline stage

_CACHE = {}


def _build(tok):
    import concourse.bacc as bacc
    import concourse.tile as tile
    from concourse import mybir

    dt = mybir.dt
    AF = mybir.ActivationFunctionType
    f32 = dt.float32
    bf = dt.bfloat16

    assert tok % TB == 0
    n_blk = tok // TB
    n_tc = TB // P  # 4 token chunks of 128 per block
    n_i = D // P    # 8 contraction chunks for mm1/mm3
    n_j = H // P    # 8 contraction chunks for mm2

    nc = bacc.Bacc(trn_type="TRN2", target_bir_lowering=False)
    x_h = nc.dram_tensor("x", [tok, D], f32, kind="ExternalInput")
    w1_h = nc.dram_tensor("w1", [D, H], f32, kind="ExternalInput")
    w2_h = nc.dram_tensor("w2", [H, D], f32, kind="ExternalInput")
    w3_h = nc.dram_tensor("w3", [D, H], f32, kind="ExternalInput")
    out_h = nc.dram_tensor("out", [tok, D], f32, kind="ExternalOutput")

    with tile.TileContext(nc) as tc:
        with (
            tc.tile_pool(name="wpool", bufs=1) as wpool,
            tc.tile_pool(name="wstp", bufs=3) as wstp,
            tc.tile_pool(name="xfpool", bufs=10) as xfpool,
            tc.tile_pool(name="xbpool", bufs=10) as xbpool,
            tc.tile_pool(name="xtpool", bufs=3) as xtpool,
            tc.tile_pool(name="htpool", bufs=16) as htpool,
            tc.tile_pool(name="spool", bufs=8) as spool,
            tc.tile_pool(name="opool", bufs=3) as opool,
            tc.tile_pool(name="psum", bufs=8, space="PSUM") as pp,
        ):
            x_r = x_h[:, :].rearrange("(b c p) d -> b c p d", p=P, c=n_tc)
            o_r = out_h[:, :].rearrange("(b c p) d -> b c p d", p=P, c=n_tc)

            x_dma, x_tiles = {}, {}

            def load_x_dma(b):
                # stage 1: HBM -> SBUF f32 + DVE bf16 convert, per tc
                xbs = []
                for t in range(n_tc):
                    xf = xfpool.tile([P, D], f32, name="xf", tag="xf")
                    nc.sync.dma_start(out=xf, in_=x_r[b, t])
                    xb = xbpool.tile([P, D], bf, name="xb", tag="xb")
                    nc.vector.tensor_copy(xb, xf)
                    xbs.append(xb)
                x_dma[b] = xbs

            def load_x_transpose(b):
                # stage 2: XBAR transpose on the scalar HWDGE queue
                xbs = x_dma.pop(b)
                xT = xtpool.tile([P, n_tc * n_i, P], bf, name="xT", tag="xT")
                for t in range(n_tc):
                    nc.scalar.dma_start(
                        out=xT[:, t * n_i:(t + 1) * n_i, :], in_=xbs[t],
                        transpose=True,
                    )
                # xT[p, tc*n_i + i, t] = x[tc*128 + t, i*128 + p]
                x_tiles[b] = xT.rearrange("p (c i) t -> p i c t", i=n_i)

            # Startup order: x0, w1 (DVE converts), x1, w3 (DVE converts),
            # w2 (ACT converts deferred into block 0).
            load_x_dma(0)
            w1b = [wpool.tile([P, H], bf, name=f"w1b{i}") for i in range(n_i)]
            w3b = [wpool.tile([P, H], bf, name=f"w3b{i}") for i in range(n_i)]
            w2b = [wpool.tile([P, D], bf, name=f"w2b{j}") for j in range(n_j)]
            for i in range(n_i):
                wst = wstp.tile([P, H], f32, name="wst", tag="wst")
                nc.sync.dma_start(out=wst, in_=w1_h[i * P:(i + 1) * P, :])
                nc.vector.tensor_copy(w1b[i], wst)
            load_x_transpose(0)
            if n_blk > 1:
                load_x_dma(1)
                load_x_transpose(1)
            for i in range(n_i):
                wst = wstp.tile([P, H], f32, name="wst", tag="wst")
                nc.sync.dma_start(out=wst, in_=w3_h[i * P:(i + 1) * P, :])
                nc.vector.tensor_copy(w3b[i], wst)
            w2st = []
            for j in range(n_j):
                wst = wstp.tile([P, D], f32, name="wst", tag="wst")
                nc.sync.dma_start(out=wst, in_=w2_h[j * P:(j + 1) * P, :])
                w2st.append(wst)

            for b in range(n_blk):
                xT = x_tiles.pop(b)

                # ---- mm1 (j loop; only needs w1) -> s1[j] = silu(x @ w1)
                s1s = []
                for j in range(n_j):
                    pA = pp.tile([P, TB], f32, name="pA", tag="ps")
                    for i in range(n_i):
                        nc.tensor.matmul(
                            pA, w1b[i][:, j * P:(j + 1) * P], xT[:, i],
                            start=(i == 0), stop=(i == n_i - 1),
                        )
                    s1 = spool.tile([P, TB], bf, name="s1", tag="s1")
                    nc.scalar.activation(s1, pA, AF.Silu)
                    s1s.append(s1)

                # x pipeline for b+2, emitted mid-block: converts/transposes
                # land behind this block's silus, ahead of its muls/evicts.
                if b + 2 < n_blk:
                    load_x_dma(b + 2)
                    load_x_transpose(b + 2)

                if b == 0:
                    # w2 converts: after block 0's silus on the scalar queue.
                    for j in range(n_j):
                        nc.scalar.activation(w2b[j], w2st[j], AF.Copy)
                    w2st = None

                # ---- mm3 (j loop; needs w3) -> hT[j] = s1[j] * (x @ w3)
                hts = []
                for j in range(n_j):
                    pB = pp.tile([P, TB], f32, name="pB", tag="ps")
                    for i in range(n_i):
                        nc.tensor.matmul(
                            pB, w3b[i][:, j * P:(j + 1) * P], xT[:, i],
                            start=(i == 0), stop=(i == n_i - 1),
                        )
                    ht = htpool.tile([P, TB], bf, name="ht", tag="ht")
                    nc.vector.tensor_mul(ht, pB, s1s[j])
                    hts.append(ht)

                # ---- mm2 -> natural-layout out, two 512-col halves
                for t in range(n_tc):
                    pC = pp.tile([P, 512], f32, name="pC", tag="ps")
                    for j in range(n_j):
                        nc.tensor.matmul(
                            pC, hts[j][:, t * P:(t + 1) * P], w2b[j][:, 0:512],
                            start=(j == 0), stop=(j == n_j - 1),
                        )
                    pD = pp.tile([P, 512], f32, name="pD", tag="ps")
                    for j in range(n_j):
                        nc.tensor.matmul(
                            pD, hts[j][:, t * P:(t + 1) * P], w2b[j][:, 512:1024],
                            start=(j == 0), stop=(j == n_j - 1),
                        )
                    o_t = opool.tile([P, D], f32, name="o_t", tag="o_t")
                    nc.scalar.activation(o_t[:, 0:512], pC, AF.Copy)
                    nc.vector.tensor_copy(o_t[:, 512:1024], pD)
                    nc.sync.dma_start(out=o_r[b, t], in_=o_t)

    nc.compile()
    return nc


def _get_nc(tok):
    if tok not in _CACHE:
        _CACHE[tok] = _build(tok)
    return _CACHE[tok]


def kernel(x, w1, w2, w3, m_sizes):
    from concourse.bass_utils import run_bass_kernel_spmd

    x = np.asarray(x, dtype=np.float32)
    w1 = np.asarray(w1, dtype=np.float32)
    w2 = np.asarray(w2, dtype=np.float32)
    w3 = np.asarray(w3, dtype=np.float32)
    sizes = np.asarray(m_sizes).astype(np.int64)
    offs = np.concatenate([[0], np.cumsum(sizes)])
    n_exp = sizes.shape[0]
    assert n_exp == N_CORES

    pad = int(max(int(sizes.max()), TB))
    pad = ((pad + TB - 1) // TB) * TB
    nc = _get_nc(pad)

    in_maps = []
    for e in range(N_CORES):
        xe = x[offs[e]:offs[e + 1]]
        if xe.shape[0] < pad:
            xe = np.concatenate(
                [xe, np.zeros((pad - xe.shape[0], D), dtype=np.float32)], axis=0
            )
        in_maps.append({"x": xe, "w1": w1[e], "w2": w2[e], "w3": w3[e]})

    r = run_bass_kernel_spmd(nc, in_maps, core_ids=list(range(N_CORES)))
    out = np.concatenate(
        [r.results[e]["out"][: sizes[e]] for e in range(N_CORES)], axis=0
    )
    return out.astype(np.float32)


# revision 12
# speedup vs baseline: 1.2416x; 1.2416x over previous
"""Grouped SwiGLU expert FFN (MoE) on 8 Trainium2 NeuronCores.

Expert parallelism: expert e's weights + its (pre-sorted) token slice go to
core e. Each core runs x@w1, x@w3, silu/mul, h@w2 for its 8192 tokens.

All matmul operands are converted to bf16 on-chip (PSUM accumulation stays
fp32; bf16 runs the PE at the same 1 row/cycle as f32r but without the
~14ns/matmul issue tax fp32r pays). The x transpose is done by the DMA XBAR
(four [128, 1024] bf16 SBUF->SBUF transposes per 512-token block), so the
PE runs matmuls only. Transpose DMAs are dispatched from the scalar (ACT)
HWDGE queue so their input-sem waits never block the sync queue's weight/x
loads; the x pipeline (DMA -> DVE bf16 convert -> XBAR transpose) runs two
blocks ahead.

Math per core (dims: t=tokens, i=dim_in, j=dim_hid, o=dim_in):
  mm1/mm3: psum[j,t] += lhsT=w{1,3}[i_chunk, j_chunk] (stationary, natural
           layout), rhs=xT[i_chunk, t_block] (moving, 512 wide) -> h1T/h3T.
  SwiGLU:  hT_j = silu(h1T) * h3T  (ACT Silu -> s1, DVE mul fuses PSUM evict;
           per-j tiles so mm2's first weight load only waits on mul(j)).
  mm2:     lhsT=hT_j[t_chunk] (stationary), rhs=w2[j_chunk, o_half]
           (moving, 512 wide) -> psum[t,o] natural-layout fp32 output.

A single 8-slot PSUM pool rotates banks through mm1/mm3/mm2 accumulators.
"""

import sys

sys.path.insert(0, "/opt/trn_rl_repo")

import numpy as np

N_CORES = 8
D = 1024  # dim_in
H = 1024  # dim_hid
P = 128
TB = 512  # token block per pipeline stage

_CACHE = {}


def _build(tok):
    import concourse.bacc as bacc
    import concourse.tile as tile
    from concourse import mybir

    dt = mybir.dt
    AF = mybir.ActivationFunctionType
    f32 = dt.float32
    bf = dt.bfloat16

    assert tok % TB == 0
    n_blk = tok // TB
    n_tc = TB // P  # 4 token chunks of 128 per block
    n_i = D // P    # 8 contraction chunks for mm1/mm3
    n_j = H // P    # 8 contraction chunks for mm2

    nc = bacc.Bacc(trn_type="TRN2", target_bir_lowering=False)
    x_h = nc.dram_tensor("x", [tok, D], f32, kind="ExternalInput")
    w1_h = nc.dram_tensor("w1", [D, H], f32, kind="ExternalInput")
    w2_h = nc.dram_tensor("w2", [H, D], f32, kind="ExternalInput")
    w3_h = nc.dram_tensor("w3", [D, H], f32, kind="ExternalInput")
    out_h = nc.dram_tensor("out", [tok, D], f32, kind="ExternalOutput")

    with tile.TileContext(nc) as tc:
        with (
            tc.tile_pool(name="wpool", bufs=1) as wpool,
            tc.tile_pool(name="wstp", bufs=3) as wstp,
            tc.tile_pool(name="xfpool", bufs=2) as xfpool,
            tc.tile_pool(name="xbpool", bufs=2) as xbpool,
            tc.tile_pool(name="xtpool", bufs=3) as xtpool,
            tc.tile_pool(name="htpool", bufs=16) as htpool,
            tc.tile_pool(name="spool", bufs=8) as spool,
            tc.tile_pool(name="opool", bufs=3) as opool,
            tc.tile_pool(name="psum", bufs=8, space="PSUM") as pp,
        ):
            x_r = x_h[:, :].rearrange("(b c p) d -> b p c d", p=P, c=n_tc)
            o_r = out_h[:, :].rearrange("(b c p) d -> b c p d", p=P, c=n_tc)

            x_dma, x_tiles = {}, {}

            def load_x_dma(b):
                # stage 1: HBM -> SBUF f32 + one DVE bf16 convert
                xf = xfpool.tile([P, n_tc, D], f32, name="xf", tag="xf")
                nc.sync.dma_start(out=xf, in_=x_r[b])
                xb = xbpool.tile([P, n_tc * D], bf, name="xb", tag="xb")
                nc.vector.tensor_copy(xb, xf.rearrange("p c d -> p (c d)"))
                x_dma[b] = xb

            def load_x_transpose(b):
                # stage 2: one [128, n_tc*D] XBAR transpose (sync HWDGE)
                xb = x_dma.pop(b)
                xT = xtpool.tile([P, n_tc * n_i, P], bf, name="xT", tag="xT")
                nc.sync.dma_start(out=xT, in_=xb, transpose=True)
                # xT[p, tc*n_i + i, t] = x[tc*128 + t, i*128 + p]
                x_tiles[b] = xT.rearrange("p (c i) t -> p i c t", i=n_i)

            # Startup order on the sync queue: x0, w1, xT0-transpose
            # (waits for the xb0 convert, but w1 is already dispatched),
            # x1, w3, xT1, w2. w1/w3 converts on DVE; w2 converts deferred
            # into block 0 (ACT, after its silus).
            load_x_dma(0)
            w1b = [wpool.tile([P, H], bf, name=f"w1b{i}") for i in range(n_i)]
            w3b = [wpool.tile([P, H], bf, name=f"w3b{i}") for i in range(n_i)]
            w2b = [wpool.tile([P, D], bf, name=f"w2b{j}") for j in range(n_j)]
            for i in range(n_i):
                wst = wstp.tile([P, H], f32, name="wst", tag="wst")
                nc.sync.dma_start(out=wst, in_=w1_h[i * P:(i + 1) * P, :])
                nc.vector.tensor_copy(w1b[i], wst)
            load_x_transpose(0)
            if n_blk > 1:
                load_x_dma(1)
            for i in range(n_i):
                wst = wstp.tile([P, H], f32, name="wst", tag="wst")
                nc.sync.dma_start(out=wst, in_=w3_h[i * P:(i + 1) * P, :])
                nc.vector.tensor_copy(w3b[i], wst)
            if n_blk > 1:
                load_x_transpose(1)
            w2st = []
            for j in range(n_j):
                wst = wstp.tile([P, D], f32, name="wst", tag="wst")
                nc.sync.dma_start(out=wst, in_=w2_h[j * P:(j + 1) * P, :])
                w2st.append(wst)

            for b in range(n_blk):
                xT = x_tiles.pop(b)

                # ---- mm1 (j loop; only needs w1) -> s1[j] = silu(x @ w1)
                s1s = []
                for j in range(n_j):
                    pA = pp.tile([P, TB], f32, name="pA", tag="ps")
                    for i in range(n_i):
                        nc.tensor.matmul(
                            pA, w1b[i][:, j * P:(j + 1) * P], xT[:, i],
                            start=(i == 0), stop=(i == n_i - 1),
                        )
                    s1 = spool.tile([P, TB], bf, name="s1", tag="s1")
                    nc.scalar.activation(s1, pA, AF.Silu)
                    s1s.append(s1)

                if b == 0:
                    # w2 converts: after block 0's silus on the scalar queue.
                    for j in range(n_j):
                        nc.scalar.activation(w2b[j], w2st[j], AF.Copy)
                    w2st = None

                # ---- mm3 (j loop; needs w3) -> hT[j] = s1[j] * (x @ w3)
                hts = []
                for j in range(n_j):
                    pB = pp.tile([P, TB], f32, name="pB", tag="ps")
                    for i in range(n_i):
                        nc.tensor.matmul(
                            pB, w3b[i][:, j * P:(j + 1) * P], xT[:, i],
                            start=(i == 0), stop=(i == n_i - 1),
                        )
                    ht = htpool.tile([P, TB], bf, name="ht", tag="ht")
                    nc.vector.tensor_mul(ht, pB, s1s[j])
                    hts.append(ht)

                # x pipeline for b+2: the DVE convert queues after this
                # block's muls; the transpose dispatch never blocks weight
                # or output DMAs (they are all already dispatched / later).
                if b + 2 < n_blk:
                    load_x_dma(b + 2)
                    load_x_transpose(b + 2)

                # ---- mm2 -> natural-layout out, two 512-col halves
                for t in range(n_tc):
                    pC = pp.tile([P, 512], f32, name="pC", tag="ps")
                    for j in range(n_j):
                        nc.tensor.matmul(
                            pC, hts[j][:, t * P:(t + 1) * P], w2b[j][:, 0:512],
                            start=(j == 0), stop=(j == n_j - 1),
                        )
                    pD = pp.tile([P, 512], f32, name="pD", tag="ps")
                    for j in range(n_j):
                        nc.tensor.matmul(
                            pD, hts[j][:, t * P:(t + 1) * P], w2b[j][:, 512:1024],
                            start=(j == 0), stop=(j == n_j - 1),
                        )
                    o_t = opool.tile([P, D], f32, name="o_t", tag="o_t")
                    nc.scalar.activation(o_t[:, 0:512], pC, AF.Copy)
                    nc.vector.tensor_copy(o_t[:, 512:1024], pD)
                    nc.sync.dma_start(out=o_r[b, t], in_=o_t)

    nc.compile()
    return nc


def _get_nc(tok):
    if tok not in _CACHE:
        _CACHE[tok] = _build(tok)
    return _CACHE[tok]


def kernel(x, w1, w2, w3, m_sizes):
    from concourse.bass_utils import run_bass_kernel_spmd

    x = np.asarray(x, dtype=np.float32)
    w1 = np.asarray(w1, dtype=np.float32)
    w2 = np.asarray(w2, dtype=np.float32)
    w3 = np.asarray(w3, dtype=np.float32)
    sizes = np.asarray(m_sizes).astype(np.int64)
    offs = np.concatenate([[0], np.cumsum(sizes)])
    n_exp = sizes.shape[0]
    assert n_exp == N_CORES

    pad = int(max(int(sizes.max()), TB))
    pad = ((pad + TB - 1) // TB) * TB
    nc = _get_nc(pad)

    in_maps = []
    for e in range(N_CORES):
        xe = x[offs[e]:offs[e + 1]]
        if xe.shape[0] < pad:
            xe = np.concatenate(
                [xe, np.zeros((pad - xe.shape[0], D), dtype=np.float32)], axis=0
            )
        in_maps.append({"x": xe, "w1": w1[e], "w2": w2[e], "w3": w3[e]})

    r = run_bass_kernel_spmd(nc, in_maps, core_ids=list(range(N_CORES)))
    out = np.concatenate(
        [r.results[e]["out"][: sizes[e]] for e in range(N_CORES)], axis=0
    )
    return out.astype(np.float32)
